# revision 24
# baseline (speedup 1.0000x reference)
"""GQA causal self-attention (sliding window 1024 + 4-token sink) on 8 trn2
NeuronCores.

Sharding: data parallel on batch (2) x tensor parallel on kv-head groups (4).
Core c handles batch c//4 and kv head c%4 (query heads 4g..4g+3): wq/wk/wv are
split column-wise (rows of the [out,in] weights), wo row-wise; each core
produces a [C,T] partial of the output projection (bf16) and the host sums the
4 partials per batch in fp32.

Per-core kernel:
  All four GEMMs (q/k/v projections and the output projection) run as fp8
  DoubleRow matmuls (0.5 PE cycles per moving row, K=256 per instruction) with
  3-term hi/lo error compensation: operands are split host-side into an e4m3
  "hi" part and an e5m2 "lo" residual (e5m2's wide exponent range keeps the
  small residuals out of subnormal territory), and x@w is computed as
  xh@wh + xl@wh + xh@wl accumulated in one PSUM group. Weights are pre-scaled
  by 2^6 (and y by 2^5) so the hi parts sit in e4m3's sweet spot; the inverse
  scales fold into the ACT evacuation and a host-side multiply.

  Attention stays bf16: scores S^T[tj,ti] per 128-wide key tile against the
  9-tile sliding window + sink, RoPE applied in [d,t] layout via a half-swap
  permutation matmul + elementwise combine, masking by zeroing exp(S^T) blocks
  in SBUF (gpsimd affine_select), softmax without max-subtraction (|scale*S|
  <= ~6 for this distribution), denominators via a ones-vector matmul riding
  the same PT stream, y^T accumulated in PSUM, column-scaled by 32/sum, then
  split (e4m3 hi via ACT, e5m2 residual via DVE) to feed the fp8 wo matmul.
"""

import os
import sys

import numpy as np
import ml_dtypes

sys.path.insert(0, "/opt/trn_rl_repo")

import orjson

import concourse.bass as bass
import concourse.tile as tile
from concourse import mybir
from concourse.bass_utils import run_bass_kernel_spmd

# ---------------------------------------------------------------------------
# Workarounds for the walrus build in this container: it rejects more than one
# sync-wait per instruction (setupSyncWait on the *_NO_STRUCT encodings).
# 1) TileContext's final drain carries one wait per live proc -> put each wait
#    on its own NoOp ahead of a clean drain.
# 2) Any scheduled instruction can end up with >1 waits -> post-process the
#    serialized BIR and hoist extra waits onto single-wait NoOps injected just
#    before the instruction on the same engine (same-engine program order makes
#    this equivalent).
# ---------------------------------------------------------------------------
import bass_rust
from bass_rust import ScopedClock


def _patched_drain_and_barrier(self, tick_clock, wait_clock):
    nop_inst = self.nc.sync.nop(nofuse=True, hint="drain_waits")
    wait_clock.add_sem_waits(
        nop_inst.ins, ScopedClock({None: tick_clock.global_clock})
    )
    si = nop_inst.ins.sync_info
    waits = list(si.on_wait) if si is not None else []
    if si is not None:
        si.on_wait = waits[:1]
    for w in waits[1:]:
        extra = self.nc.sync.nop(nofuse=True, hint="drain_waits")
        extra.ins.sync_info = bass_rust.SyncInfo(on_wait=[w], on_update=[])
    self.nc.sync.drain()
    self.nc.all_engine_barrier()
    assert self.sems is not None
    popped = self.nc._tile_sem_poison_stack.pop()
    assert popped is self._sem_poison
    self.nc.clear_and_free_semaphores(list(self.sems.allocated().values()))
    self.nc.all_engine_barrier()


tile.TileContext._drain_and_barrier = _patched_drain_and_barrier

_orig_to_json_bytes = bass.Bass.to_json_bytes
_WSPLIT_COUNTER = [0]


def _split_multi_waits(mod: dict) -> dict:
    for fn in mod.get("functions", []):
        for blk in fn.get("blocks", []):
            insts = blk.get("instructions")
            if not insts:
                continue
            new_insts = []
            changed = False
            for inst in insts:
                si = inst.get("sync_info") or {}
                waits = si.get("on_wait") or []
                if len(waits) > 1:
                    changed = True
                    for w in waits:
                        _WSPLIT_COUNTER[0] += 1
                        new_insts.append({
                            "name": f"I-wsplit-{_WSPLIT_COUNTER[0]}",
                            "opcode": "NoOp",
                            "engine": inst["engine"],
                            "ins": [],
                            "outs": [],
                            "debug": inst.get("debug"),
                            "sync_info": {"on_wait": [w], "on_update": []},
                        })
                    si = dict(si)
                    si["on_wait"] = []
                    inst = dict(inst)
                    inst["sync_info"] = si
                new_insts.append(inst)
            if changed:
                blk["instructions"] = new_insts
    return mod


def _patched_to_json_bytes(self) -> bytes:
    mod = orjson.loads(_orig_to_json_bytes(self))
    return orjson.dumps(_split_multi_waits(mod))


bass.Bass.to_json_bytes = _patched_to_json_bytes

# ---------------------------------------------------------------------------
# Problem constants (hardcoded per the task contract).
# ---------------------------------------------------------------------------
B, T, C = 2, 2048, 2048
N_HEAD, N_KV, D = 16, 4, 128
WINDOW, SINK, THETA = 1024, 4, 10000.0
SCALE = 1.0 / float(np.sqrt(D))
N_CORES = 8
HPG = N_HEAD // N_KV          # query heads per kv group (4)
NT = T // 128                 # 16 query/key tiles
NCK = C // 256                # 8 contraction chunk-pairs (K=256 each)
BF = mybir.dt.bfloat16
F16 = mybir.dt.float16
F32 = mybir.dt.float32
E4 = mybir.dt.float8e4
E5 = mybir.dt.float8e5
DR = mybir.MatmulPerfMode.DoubleRow
W_SCALE = 2.0 ** -6           # weights pre-scaled x64 on the host
Y_SCALE = 32.0                # y scaled x32 before the fp8 split
OUT_DESCALE = 2.0 ** -11      # host-side: undo 2^6 (wo) * 2^5 (y)

LAST_RESULT = None            # test harness reads exec_time_ns off this


def _half_kjs(H):
    """Key tiles feeding query half H (8 query tiles). The first entry covers
    the FULL half (kj=0 for H=0 via the window; kj=8 for H=1 via the window)
    so every PSUM accumulation starts there; for H=1 the kj=0 sink/edge tile
    comes second so its exp + mask latency hides behind kj=8's big matmuls."""
    starter = 8 * H
    out = [(starter, 8 * H, 8 * H + 7)]
    for kj in range(NT):
        if kj == starter:
            continue
        if kj == 0:
            # sink tile: visible to the whole upper half (bsmask prunes rows)
            out.append((0, 8 * H, 8 * H + 7))
            continue
        lo, hi = max(kj, 8 * H), min(kj + 8, 8 * H + 7)
        if lo <= hi:
            out.append((kj, lo, hi))
    return out


_PHASES = 3


def _build_program(n_loop=1):
    nc = bass.Bass("TRN2", target_bir_lowering=False, debug=False,
                   num_devices=N_CORES)

    # fp8 operands arrive pre-packed for DoubleRow: contraction row
    # 256*ck + 128*j + p lives at [p, 2*ck + j, :].
    xh_d = nc.declare_dram_parameter("xh", [128, 2 * NCK, T], E4, isOutput=False)
    xl_d = nc.declare_dram_parameter("xl", [128, 2 * NCK, T], E5, isOutput=False)
    wqh_d = nc.declare_dram_parameter("wqh", [128, 2 * NCK, HPG * D], E4,
                                      isOutput=False)
    wql_d = nc.declare_dram_parameter("wql", [128, 2 * NCK, HPG * D], E5,
                                      isOutput=False)
    wkh_d = nc.declare_dram_parameter("wkh", [128, 2 * NCK, D], E4, isOutput=False)
    wkl_d = nc.declare_dram_parameter("wkl", [128, 2 * NCK, D], E5, isOutput=False)
    wvh_d = nc.declare_dram_parameter("wvh", [128, 2 * NCK, D], E4, isOutput=False)
    wvl_d = nc.declare_dram_parameter("wvl", [128, 2 * NCK, D], E5, isOutput=False)
    # wo rows (the head dim, K=512) packed the same way: row 256*p + 128*j + q
    # at [q, 2*p + j, :].
    woh_d = nc.declare_dram_parameter("woh", [128, 4, T], E4, isOutput=False)
    wol_d = nc.declare_dram_parameter("wol", [128, 4, T], E5, isOutput=False)
    cc_d = nc.declare_dram_parameter("cc", [D, T], BF, isOutput=False)
    ss_d = nc.declare_dram_parameter("ss", [D, T], BF, isOutput=False)
    r_d = nc.declare_dram_parameter("rmat", [D, D], BF, isOutput=False)
    id_d = nc.declare_dram_parameter("ident", [D, D], BF, isOutput=False)
    # 0/1 mask for the kj=0 blocks of the upper query half: block 0 is the
    # window-edge-or-sink pattern for q-tile 8, blocks 1..7 are sink-rows-only.
    bs_d = nc.declare_dram_parameter("bsmask", [D, 1024], BF, isOutput=False)
    outT_d = nc.declare_dram_parameter("outT", [C, T], BF, isOutput=True)

    NCHUNK = T // 512  # 4

    def _emit_body(tc):
        with tc.tile_pool(name="consts", bufs=1) as consts, \
             tc.tile_pool(name="persist", bufs=1) as persist:

            cc_sb = consts.tile([D, T], BF, tag="cc", name="cc")
            ss_sb = consts.tile([D, T], BF, tag="ss", name="ss")
            r_sb = consts.tile([D, D], BF, tag="rmat", name="rmat")
            id_sb = consts.tile([D, D], BF, tag="ident", name="ident")
            ones_col = consts.tile([128, 1], BF, tag="ones_col", name="ones_col")
            ones_row = consts.tile([1, 128], F16, tag="ones_row", name="ones_row")
            bs_sb = consts.tile([D, 1024], BF, tag="bsmask", name="bsmask")
            woh = consts.tile([128, 4, T], E4, tag="woh", name="woh")
            wol = consts.tile([128, 4, T], E5, tag="wol", name="wol")

            qT = [persist.tile([128, T], BF, tag=f"qT{h}", name=f"qT{h}")
                  for h in range(HPG)]
            kT = persist.tile([128, T], BF, tag="kT", name="kT")
            vT_raw = persist.tile([128, T], BF, tag="vT_raw", name="vT_raw")
            v_nat = persist.tile([128, T], BF, tag="v_nat", name="v_nat")
            # normalized y (x32), split hi/lo, packed per head-pair for the
            # K=512 wo contraction: head 2p+j lives at [:, j, :] of pair p.
            yh = [persist.tile([128, 2, T], E4, tag=f"yh{p}", name=f"yh{p}")
                  for p in range(2)]
            yl = [persist.tile([128, 2, T], E5, tag=f"yl{p}", name=f"yl{p}")
                  for p in range(2)]

            # ================= projections (fp8 DoubleRow 3-term) ==========
            # T-block-outer: x streams in 512-column blocks; per block the
            # six output units (k, v, q0..q3) each accumulate their 24 DR
            # matmuls into one rolling PSUM tile, so the PE starts ~3us in
            # and never waits on staging again.
            with tc.tile_pool(name="stage", bufs=1) as stage, \
                 tc.tile_pool(name="xblk", bufs=3) as xblk:
                wqh_sb = stage.tile([128, 2 * NCK, HPG * D], E4, tag="wqh",
                                    name="wqh")
                wql_sb = stage.tile([128, 2 * NCK, HPG * D], E5, tag="wql",
                                    name="wql")
                wkh_sb = stage.tile([128, 2 * NCK, D], E4, tag="wkh", name="wkh")
                wkl_sb = stage.tile([128, 2 * NCK, D], E5, tag="wkl", name="wkl")
                wvh_sb = stage.tile([128, 2 * NCK, D], E4, tag="wvh", name="wvh")
                wvl_sb = stage.tile([128, 2 * NCK, D], E5, tag="wvl", name="wvl")

                xtiles = {}

                def stage_block(tb, xh_only=False, xl_only=False):
                    if not xl_only:
                        xh_t = xblk.tile([128, 2 * NCK, 512], E4, tag="xh",
                                         name=f"xh{tb}")
                        nc.sync.dma_start(
                            out=xh_t, in_=xh_d[:, :, 512 * tb:512 * tb + 512])
                        xtiles[tb] = (xh_t, None)
                    if not xh_only:
                        xl_t = xblk.tile([128, 2 * NCK, 512], E5, tag="xl",
                                         name=f"xl{tb}")
                        nc.sync.dma_start(
                            out=xl_t, in_=xl_d[:, :, 512 * tb:512 * tb + 512])
                        xtiles[tb] = (xtiles[tb][0], xl_t)

                # block 0 split along the chunk-pair dim (keeps 512B
                # descriptor runs) so the k unit starts on the first half
                xh0 = xblk.tile([128, 2 * NCK, 512], E4, tag="xh", name="xh0")
                xl0 = xblk.tile([128, 2 * NCK, 512], E5, tag="xl", name="xl0")
                xtiles[0] = (xh0, xl0)
                nc.sync.dma_start(out=xh0[:, 0:4, :], in_=xh_d[:, 0:4, 0:512])
                nc.sync.dma_start(out=wkh_sb, in_=wkh_d[:, :, :])
                nc.sync.dma_start(out=xh0[:, 4:NCK, :],
                                  in_=xh_d[:, 4:NCK, 0:512])
                nc.sync.dma_start(out=wvh_sb, in_=wvh_d[:, :, :])
                nc.sync.dma_start(out=xh0[:, NCK:2 * NCK, :],
                                  in_=xh_d[:, NCK:2 * NCK, 0:512])
                nc.sync.dma_start(out=wkl_sb, in_=wkl_d[:, :, :])
                nc.sync.dma_start(out=wvl_sb, in_=wvl_d[:, :, :])
                nc.sync.dma_start(out=wqh_sb, in_=wqh_d[:, :, :])
                nc.sync.dma_start(out=xl0, in_=xl_d[:, :, 0:512])
                nc.sync.dma_start(out=wql_sb, in_=wql_d[:, :, :])
                stage_block(1)
                nc.sync.dma_start(out=cc_sb, in_=cc_d[:, :])
                nc.sync.dma_start(out=ss_sb, in_=ss_d[:, :])
                nc.sync.dma_start(out=r_sb, in_=r_d[:, :])
                nc.sync.dma_start(out=id_sb, in_=id_d[:, :])
                nc.vector.memset(ones_col, 1.0)
                nc.vector.memset(ones_row, Y_SCALE)

                with tc.tile_pool(name="proj_ps", bufs=6, space="PSUM") as pps, \
                     tc.tile_pool(name="rope_ps", bufs=1, space="PSUM") as rps, \
                     tc.tile_pool(name="vt_ps", bufs=1, space="PSUM") as vps, \
                     tc.tile_pool(name="rope_sb", bufs=6) as rsb:

                    pending = []  # deferred post-processing closures

                    def make_post(unit, c0, raw):
                        def post():
                            if unit[0] == "v":
                                vslice = vT_raw[:, c0:c0 + 512]
                                nc.vector.tensor_copy(vslice, raw)
                                for j in range(4):
                                    tp = vps.tile([128, 128], BF, tag="vt",
                                                  name="vt")
                                    nc.tensor.transpose(
                                        tp,
                                        vT_raw[:, c0 + 128 * j:c0 + 128 * j + 128],
                                        id_sb)
                                    nc.vector.tensor_copy(
                                        v_nat[:, c0 + 128 * j:c0 + 128 * j + 128],
                                        tp)
                            else:
                                dst = kT if unit[0] == "k" else qT[unit[1]]
                                # t1 on the otherwise-idle Pool engine keeps
                                # the DVE from backlogging at proj end
                                t1 = rsb.tile([128, 512], BF, tag="t1", name="t1")
                                nc.gpsimd.tensor_mul(t1, raw, cc_sb[:, c0:c0 + 512])
                                rot = rps.tile([128, 512], F32, tag="rot",
                                               name="rot")
                                nc.tensor.matmul(rot, r_sb, raw,
                                                 start=True, stop=True)
                                t2 = rsb.tile([128, 512], BF, tag="t2", name="t2")
                                nc.vector.tensor_mul(t2, rot, ss_sb[:, c0:c0 + 512])
                                nc.vector.tensor_add(dst[:, c0:c0 + 512], t1, t2)
                        return post

                    units = [(("k",), wkh_sb, wkl_sb, 0),
                             (("v",), wvh_sb, wvl_sb, 0)] + \
                            [(("q", h), wqh_sb, wql_sb, 128 * h)
                             for h in range(HPG)]

                    def emit_term(ps, wh, wl, col, tb, term, first, last):
                        xh_t, xl_t = xtiles[tb]
                        for ck in range(NCK):
                            ws = (wh if term != 1 else wl)[
                                :, 2 * ck:2 * ck + 2, col:col + 128]
                            xs = (xh_t if term != 2 else xl_t)[
                                :, 2 * ck:2 * ck + 2, :]
                            nc.tensor.matmul(
                                ps, ws, xs,
                                start=(first and ck == 0),
                                stop=(last and ck == NCK - 1),
                                perf_mode=DR)

                    def evac(unit, ps, tb):
                        raw = rsb.tile([128, 512], BF, tag="raw", name="raw")
                        nc.scalar.activation(
                            raw, ps, mybir.ActivationFunctionType.Copy,
                            bias=0.0, scale=W_SCALE)
                        pending.append(make_post(unit, 512 * tb, raw))

                    # Block 0: terms ordered by operand DMA arrival across
                    # units (hi terms as x-hi lands, k/v w-lo next, q hi
                    # before x-lo, w-lo last) with all six PSUM groups open.
                    b0ps = {unit: pps.tile([128, 512], F32, tag="proj",
                                           name="proj")
                            for unit, _, _, _ in units}
                    for unit, wh, wl, col in units[:2]:
                        emit_term(b0ps[unit], wh, wl, col, 0, 0, True, False)
                        emit_term(b0ps[unit], wh, wl, col, 0, 1, False, False)
                    for unit, wh, wl, col in units[2:]:
                        emit_term(b0ps[unit], wh, wl, col, 0, 0, True, False)
                    for unit, wh, wl, col in units[:2]:
                        emit_term(b0ps[unit], wh, wl, col, 0, 2, False, True)
                        evac(unit, b0ps[unit], 0)
                    for unit, wh, wl, col in units[2:]:
                        emit_term(b0ps[unit], wh, wl, col, 0, 2, False, False)
                    for unit, wh, wl, col in units[2:]:
                        emit_term(b0ps[unit], wh, wl, col, 0, 1, False, True)
                        evac(unit, b0ps[unit], 0)
                        while len(pending) > 2:
                            pending.pop(0)()

                    stage_block(2)
                    for tb in range(1, NCHUNK):
                        if tb + 2 < NCHUNK:
                            stage_block(tb + 2)
                        for unit, wh, wl, col in units:
                            ps = pps.tile([128, 512], F32, tag="proj",
                                          name="proj")
                            for term in range(3):
                                emit_term(ps, wh, wl, col, tb, term,
                                          term == 0, term == 2)
                            evac(unit, ps, tb)
                            # drain deeper on the last block so the rope
                            # chains don't pile up past the proj phase
                            depth = 2 if tb < NCHUNK - 1 else 1
                            while len(pending) > depth:
                                pending.pop(0)()
                    while pending:
                        pending.pop(0)()
                    nc.sync.dma_start(out=bs_sb, in_=bs_d[:, :])
                    nc.sync.dma_start(out=woh, in_=woh_d[:, :, :])
                    nc.sync.dma_start(out=wol, in_=wol_d[:, :, :])

            # ========================= attention ===========================
            if _PHASES < 2:
                return
            with tc.tile_pool(name="st_ps", bufs=2, space="PSUM") as sps, \
                 tc.tile_pool(name="yt_ps", bufs=1, space="PSUM") as yps, \
                 tc.tile_pool(name="cs_ps", bufs=1, space="PSUM") as cps, \
                 tc.tile_pool(name="pt_sb", bufs=4) as ptp, \
                 tc.tile_pool(name="ytu_sb", bufs=2) as ytup, \
                 tc.tile_pool(name="ytn_sb", bufs=2) as ytn, \
                 tc.tile_pool(name="norm_sb", bufs=2) as nrm:

                # Deferred work queues are GLOBAL across (head, half) pairs:
                # the previous half's colsum/AV pops (and its PSUM-evacuation
                # closure) execute interleaved with the next half's score
                # matmuls, so the PE has work during each half's exp-latency
                # pipeline fill, and tails (normalization + fp8 y-split) run
                # one half later still.
                pend = []   # deferred colsum+AV (+ evacuation closures)
                tails = []  # deferred normalization/y-split chains

                for h in range(HPG):
                    for H in range(2):
                        q0 = 1024 * H
                        kjs = _half_kjs(H)
                        first_kj = kjs[0][0]
                        last_kj = kjs[-1][0]
                        yt = yps.tile([128, 1024], F32, tag="yt", name="yt")
                        cs = cps.tile([1, 1024], F32, tag="cs", name="cs")

                        for kj, lo, hi in kjs:
                            c0, c1 = lo * 128, (hi + 1) * 128
                            ncols = c1 - c0
                            st = sps.tile([128, 1024], F32, tag="st", name="st")
                            for off in range(0, ncols, 512):
                                w = min(512, ncols - off)
                                nc.tensor.matmul(
                                    st[:, off:off + w],
                                    kT[:, 128 * kj:128 * kj + 128],
                                    qT[h][:, c0 + off:c0 + off + w],
                                    start=True, stop=True)
                            pt = ptp.tile([128, 1024], BF, tag="pt", name="pt")
                            nc.scalar.activation(
                                pt[:, :ncols], st[:, :ncols],
                                mybir.ActivationFunctionType.Exp,
                                bias=0.0, scale=SCALE)
                            # --- masks: zero disallowed entries of exp ---
                            if lo <= kj <= hi:
                                s = (kj - lo) * 128  # causal diag: keep c >= p
                                nc.gpsimd.affine_select(
                                    out=pt[:, s:s + 128], in_=pt[:, s:s + 128],
                                    compare_op=mybir.AluOpType.is_ge,
                                    fill=0.0, base=0,
                                    pattern=[[1, 128]], channel_multiplier=-1)
                            if kj >= 1 and hi == kj + 8:
                                s = (hi - lo) * 128  # window edge: keep p >= c
                                nc.gpsimd.affine_select(
                                    out=pt[:, s:s + 128], in_=pt[:, s:s + 128],
                                    compare_op=mybir.AluOpType.is_ge,
                                    fill=0.0, base=0,
                                    pattern=[[-1, 128]], channel_multiplier=1)
                            if kj == 0 and H == 1:
                                # q-tile 8: keep (p >= c) | (p < 4);
                                # q-tiles 9..15: sink rows only. One 0/1 mask.
                                nc.vector.tensor_mul(pt[:, 0:1024],
                                                     pt[:, 0:1024], bs_sb)

                            def make_post(kj, lo, hi, pt, yt, cs, first_kj,
                                          last_kj, q0):
                                c0, c1 = lo * 128, (hi + 1) * 128
                                ncols = c1 - c0
                                l0 = c0 - q0

                                def post():
                                    for off in range(0, ncols, 512):
                                        w = min(512, ncols - off)
                                        nc.tensor.matmul(
                                            cs[:, l0 + off:l0 + off + w],
                                            ones_col, pt[:, off:off + w],
                                            start=(kj == first_kj),
                                            stop=(kj == last_kj),
                                            skip_group_check=True)
                                    for off in range(0, ncols, 512):
                                        w = min(512, ncols - off)
                                        nc.tensor.matmul(
                                            yt[:, l0 + off:l0 + off + w],
                                            v_nat[:, 128 * kj:128 * kj + 128],
                                            pt[:, off:off + w],
                                            start=(kj == first_kj),
                                            stop=(kj == last_kj),
                                            skip_group_check=True)
                                return post
                            pend.append(make_post(kj, lo, hi, pt, yt, cs,
                                                  first_kj, last_kj, q0))
                            if len(pend) > 2:
                                pend.pop(0)()
                            if kj == kjs[2][0] and tails:
                                # two kjs past the starter: gives the DVE
                                # reciprocal time to finish before the PE hits
                                # the broadcast outer-product
                                tails.pop(0)()

                        def make_evac(h, H, q0, yt, cs):
                            def evac():
                                # Free the PSUM accumulators. recip first: the
                                # tail's PE broadcast only needs recip, so it
                                # unblocks a DVE-op earlier. fp16 keeps ~10
                                # mantissa bits on the denominators.
                                recip = nrm.tile([1, 1024], F16, tag="recip",
                                                 name="recip")
                                with nc.allow_low_precision(
                                        reason="fp16 1/denom: 10 mantissa "
                                               "bits on a well-scaled sum"):
                                    nc.vector.reciprocal(recip, cs)
                                ytu = ytup.tile([128, 1024], BF, tag="ytu",
                                                name="ytu")
                                nc.vector.tensor_copy(ytu, yt)

                                def tail():
                                    # 512-chunked so the last halves' chains
                                    # pipeline instead of serializing 1024
                                    # wide on the DVE
                                    rb_ps = sps.tile([128, 1024], F32,
                                                     tag="st", name="st")
                                    p, j = divmod(h, 2)
                                    yfull = ytn.tile([128, 1024], BF, tag="yf",
                                                     name="yf")
                                    for off in (0, 512):
                                        # ones_row holds Y_SCALE: rb=32/denom
                                        nc.tensor.matmul(
                                            rb_ps[:, off:off + 512],
                                            ones_row,
                                            recip[:, off:off + 512],
                                            start=True, stop=True)
                                        nc.vector.tensor_mul(
                                            yfull[:, off:off + 512],
                                            ytu[:, off:off + 512],
                                            rb_ps[:, off:off + 512])
                                        nc.scalar.activation(
                                            yh[p][:, j, q0 + off:q0 + off + 512],
                                            yfull[:, off:off + 512],
                                            mybir.ActivationFunctionType.Copy,
                                            bias=0.0, scale=1.0)
                                        nc.vector.tensor_sub(
                                            yl[p][:, j, q0 + off:q0 + off + 512],
                                            yfull[:, off:off + 512],
                                            yh[p][:, j, q0 + off:q0 + off + 512])
                                tails.append(tail)
                            return evac
                        pend.append(make_evac(h, H, q0, yt, cs))
                while pend:
                    pend.pop(0)()
                while tails:
                    tails.pop(0)()

            # ===================== output projection =======================
            if _PHASES < 3:
                return
            # Units of (row-tile o, T-half) = 2 psum chunks each; pair-0
            # matmuls run 2 units ahead of pair-1 so the first units' pair-0
            # work overlaps the last heads' normalization tails (which
            # produce pair 1), and nothing head-blocks the PE queue.
            with tc.tile_pool(name="wo_ps", bufs=6, space="PSUM") as wps, \
                 tc.tile_pool(name="out_sb", bufs=3) as osb:
                flip = 0

                def emit_p0(o, half):
                    pss = []
                    for n in (2 * half, 2 * half + 1):
                        ps = wps.tile([128, 512], F32, tag="wo", name="wo")
                        pss.append(ps)
                        whs = woh[:, 0:2, 128 * o:128 * o + 128]
                        wls = wol[:, 0:2, 128 * o:128 * o + 128]
                        yhs = yh[0][:, :, 512 * n:512 * n + 512]
                        yls = yl[0][:, :, 512 * n:512 * n + 512]
                        nc.tensor.matmul(ps, whs, yhs, start=True,
                                         stop=False, perf_mode=DR)
                        nc.tensor.matmul(ps, wls, yhs, start=False,
                                         stop=False, perf_mode=DR)
                        nc.tensor.matmul(ps, whs, yls, start=False,
                                         stop=False, perf_mode=DR)
                    return pss

                def finish(o, half, pss):
                    nonlocal flip
                    ob = osb.tile([128, 1024], BF, tag="ob", name="ob")
                    for i, n in enumerate((2 * half, 2 * half + 1)):
                        ps = pss[i]
                        whs = woh[:, 2:4, 128 * o:128 * o + 128]
                        wls = wol[:, 2:4, 128 * o:128 * o + 128]
                        yhs = yh[1][:, :, 512 * n:512 * n + 512]
                        yls = yl[1][:, :, 512 * n:512 * n + 512]
                        nc.tensor.matmul(ps, whs, yhs, start=False,
                                         stop=False, perf_mode=DR)
                        nc.tensor.matmul(ps, wls, yhs, start=False,
                                         stop=False, perf_mode=DR)
                        nc.tensor.matmul(ps, whs, yls, start=False,
                                         stop=True, perf_mode=DR)
                        dst = ob[:, 512 * i:512 * i + 512]
                        if flip % 2 == 0:
                            nc.scalar.copy(dst, ps)
                        else:
                            nc.vector.tensor_copy(dst, ps)
                        flip += 1
                    nc.sync.dma_start(
                        out=outT_d[128 * o:128 * o + 128,
                                   1024 * half:1024 * half + 1024],
                        in_=ob)

                open_q = []
                for i, (o, half) in enumerate(
                        (o, half) for o in range(NT) for half in range(2)):
                    open_q.append((o, half, emit_p0(o, half)))
                    # lookahead 2 only while the last attention tails drain;
                    # then run units serially so the end-of-kernel tail is a
                    # single unit's copy+DMA chain
                    depth = 2 if i < 6 else 0
                    while len(open_q) > depth:
                        finish(*open_q.pop(0))
                while open_q:
                    finish(*open_q.pop(0))
    with tile.TileContext(nc) as tc:
        if n_loop > 1:
            with tc.For_i(0, n_loop, 1):
                _emit_body(tc)
        else:
            _emit_body(tc)
    return nc


_PROGRAM = None


def _get_program():
    global _PROGRAM
    if _PROGRAM is None:
        _PROGRAM = _build_program()
    return _PROGRAM


def _pack_pairs(a):
    """[K, M] contraction-major -> [128, K//128, M] DoubleRow layout
    (row 256*ck + 128*j + p -> [p, 2*ck + j, :])."""
    K, M = a.shape
    return np.ascontiguousarray(
        a.reshape(K // 128, 128, M).transpose(1, 0, 2))


def _split8(a, scale=1.0):
    """Return (hi e4m3, lo e5m2) DoubleRow-packed copies of a*scale."""
    e4 = ml_dtypes.float8_e4m3
    e5 = ml_dtypes.float8_e5m2
    s = (a * scale).astype(np.float32)
    hi = s.astype(e4)
    lo = (s - hi.astype(np.float32)).astype(e5)
    return _pack_pairs(hi), _pack_pairs(lo)


def _host_inputs(x, wq, wk, wv, wo):
    bf = ml_dtypes.bfloat16
    inv_freq = 1.0 / (THETA ** (np.arange(0, D, 2, dtype=np.float32) / D))
    ang = np.outer(np.arange(T, dtype=np.float32), inv_freq)  # [T, 64]
    cosT, sinT = np.cos(ang).T, np.sin(ang).T                 # [64, T]
    cc = np.ascontiguousarray(np.concatenate([cosT, cosT], 0).astype(bf))
    ss = np.ascontiguousarray(np.concatenate([-sinT, sinT], 0).astype(bf))
    rmat = np.zeros((D, D), np.float32)
    rmat[np.arange(64), np.arange(64) + 64] = 1.0
    rmat[np.arange(64) + 64, np.arange(64)] = 1.0
    rmat = rmat.astype(bf)
    ident = np.eye(D, dtype=np.float32).astype(bf)
    p = np.arange(128)[:, None]
    c = np.arange(128)[None, :]
    bsmask = np.zeros((128, 1024), np.float32)
    bsmask[:, 0:128] = ((p >= c) | (p < SINK)).astype(np.float32)
    bsmask[0:SINK, 128:1024] = 1.0
    bsmask = np.ascontiguousarray(bsmask.astype(bf))

    x_by_batch = []
    for b in range(B):
        xh, xl = _split8(np.ascontiguousarray(x[b].T))  # [C, T]
        x_by_batch.append({"xh": xh, "xl": xl})
    w_by_group = []
    for g in range(HPG):
        wqh, wql = _split8(wq[512 * g:512 * g + 512, :].T, 1.0 / W_SCALE)
        wkh, wkl = _split8(wk[128 * g:128 * g + 128, :].T, 1.0 / W_SCALE)
        wvh, wvl = _split8(wv[128 * g:128 * g + 128, :].T, 1.0 / W_SCALE)
        woh, wol = _split8(wo[:, 512 * g:512 * g + 512].T, 1.0 / W_SCALE)
        w_by_group.append({
            "wqh": wqh, "wql": wql, "wkh": wkh, "wkl": wkl,
            "wvh": wvh, "wvl": wvl, "woh": woh, "wol": wol,
        })
    in_maps = []
    for core in range(N_CORES):
        b, g = divmod(core, HPG)
        in_maps.append({
            **x_by_batch[b], **w_by_group[g],
            "cc": cc, "ss": ss, "rmat": rmat, "ident": ident,
            "bsmask": bsmask,
        })
    return in_maps


def kernel(x, wq, wk, wv, wo):
    global LAST_RESULT
    x = np.asarray(x, np.float32)
    wq = np.asarray(wq, np.float32)
    wk = np.asarray(wk, np.float32)
    wv = np.asarray(wv, np.float32)
    wo = np.asarray(wo, np.float32)

    nc = _get_program()
    in_maps = _host_inputs(x, wq, wk, wv, wo)
    # NTFF tracing is not available under this container's axon build
    # (antenv.axon_hooks absent) and would crash run_bass_kernel_spmd.
    os.environ["BASS_NEVER_TRACE"] = "1"
    res = run_bass_kernel_spmd(nc, in_maps, list(range(N_CORES)), trace=False)
    LAST_RESULT = res

    out = np.zeros((B, T, C), np.float32)
    for core in range(N_CORES):
        b = core // HPG
        out[b] += np.asarray(res.results[core]["outT"], np.float32).T
    out *= OUT_DESCALE
    return out
